# revision 7
# baseline (speedup 1.0000x reference)
"""Trainium2 Bass kernel for a LoRA-augmented relu-gated MLP.

Math (per reference):
    y1 = x @ w_gate + b_gate + (x @ Ag) @ Bg
    y2 = x @ w_up   + b_up   + (x @ Au) @ Bu
    x3 = relu(y1) * y2
    y3 = x3 @ w_down + b_down + (x3 @ Ad) @ Bd

Strategy:
  * Host folds every LoRA pair into its base matrix (W_eff = W + A@B in
    float64) and rounds weights + activations to bf16 so the device kernel
    is a plain gated MLP running bf16 matmuls with f32 PSUM accumulation.
    bf16 stationary operands enable the PE's Fast Weight Load path, which
    hides the per-matmul LDWEIGHTS under the 512-column stream (fp32
    weights load in two passes and serialize ~60ns on every matmul).
  * Data parallel over the 8 NeuronCores: 8192 tokens -> 1024 per core,
    every core holds the full (folded) weights. Measured ~94% MFU; the
    matmul stream sits at the 216ns/512-column PE floor.
  * Per core the MLP is computed in f-quarters: gate/up produce x3T
    stripes [128f, NT] (bf16) in SBUF; the down projection consumes them
    as stationary operands and accumulates partial y3 into an SBUF-resident
    f32 accumulator; b_down is added on the first quarter's eviction. The
    last quarter streams finished y chunks straight to DRAM.
  * DMA streams are segregated by ring so none head-of-line-blocks
    another: xT + down-proj weights on the SP HWDGE ring, gate/up weight
    stream + biases on SWDGE, y writeback on the ACT HWDGE ring (idle
    during down phases). Small transfers are batched to keep per-partition
    lines >= 2KB, and the first xT batches use a staircase so the PE
    starts early and the HAM clock-gate warms without interruption.
"""

import sys
import types

import numpy as np

# The trimmed container's `antenv` lacks `axon_hooks`; bass_utils imports it
# unconditionally when tracing is requested (e.g. BASS_TRACE=1). Provide the
# degraded no-hook module so tracing falls back gracefully instead of crashing.
try:
    import antenv.axon_hooks  # noqa: F401
except ImportError:
    _m = types.ModuleType("antenv.axon_hooks")
    _m._hook = None
    _m.set_axon_ntff_profile_hook = lambda h: setattr(_m, "_hook", h)
    _m.get_axon_ntff_profile_hook = lambda: _m._hook
    sys.modules["antenv.axon_hooks"] = _m

import concourse.bacc as bacc
import concourse.bass as bass
import concourse.mybir as mybir
import concourse.tile as tile
from concourse.bass_utils import run_bass_kernel_spmd

P = 128
F32 = mybir.dt.float32
BF16 = mybir.dt.bfloat16
NP_BF16 = mybir.dt.np(mybir.dt.bfloat16)
AF = mybir.ActivationFunctionType
ALU = mybir.AluOpType


class Cfg:
    def __init__(self, nt=1024, d=2048, f=8192, fq=4, n_cores=8):
        assert nt % P == 0 and d % P == 0 and f % P == 0
        self.NT = nt          # tokens per core
        self.D = d            # model dim
        self.F = f            # ffn dim
        self.KC = d // P      # contraction chunks for gate/up
        self.NF = f // P      # f-tiles
        self.FQ = fq          # f quarters (x3T resident per quarter)
        assert self.NF % fq == 0
        self.SQ = self.NF // fq
        self.MH = min(512, nt)          # moving-dim chunk for gate/up
        self.NMH = nt // self.MH
        self.DC = min(512, d)           # down-proj d chunk
        self.ND = d // self.DC
        self.NM = nt // P               # token chunks of 128
        self.MG = 4                     # psum group size for down-proj
        self.NMG = self.NM // self.MG
        self.N_CORES = n_cores


def build_bass(cfg: Cfg):
    """Builds the per-core Bass program (same program on all cores)."""
    c = cfg
    nc = bacc.Bacc("TRN2", target_bir_lowering=False, debug=False,
                   num_swdge_queues=4)

    xt = nc.dram_tensor("xt", [P, c.NMH, c.KC, c.MH], BF16, kind="ExternalInput")
    wg = nc.dram_tensor("wg", [c.NF, P, c.KC, P], BF16, kind="ExternalInput")
    wu = nc.dram_tensor("wu", [c.NF, P, c.KC, P], BF16, kind="ExternalInput")
    wd = nc.dram_tensor("wd", [c.ND, P, c.NF, c.DC], BF16, kind="ExternalInput")
    bg = nc.dram_tensor("bg", [P, c.NF], F32, kind="ExternalInput")
    bu = nc.dram_tensor("bu", [P, c.NF], F32, kind="ExternalInput")
    bd = nc.dram_tensor("bd", [P, c.D], F32, kind="ExternalInput")
    y = nc.dram_tensor("y", [c.NT, c.D], F32, kind="ExternalOutput")

    with tile.TileContext(nc) as tc:
        with (
            tc.tile_pool(name="consts", bufs=1) as consts,
            tc.tile_pool(name="wpool", bufs=4) as wpool,
            tc.tile_pool(name="wdpool", bufs=3) as wdpool,
            tc.tile_pool(name="xTp", bufs=1) as xTp,
            tc.tile_pool(name="x3p", bufs=1) as x3p,
            tc.tile_pool(name="yp", bufs=1) as yp,
            tc.tile_pool(name="actp", bufs=2) as actp,
            tc.tile_pool(name="outp", bufs=6) as outp,
            tc.tile_pool(name="pall", bufs=1, space="PSUM") as pall,
        ):
            # the gate/up weight stream rides SWDGE (gpsimd) queues; the
            # SP HWDGE ring carries xT and the down-proj wd batches so
            # neither stream head-of-line-blocks the other. The first two
            # f-tiles' weights go on HWDGE rings instead (SWDGE's Q7 takes
            # several us to emit its first descriptors).
            def load_w(ft, eng=None):
                wgt = wpool.tile([P, c.KC, P], BF16, tag="w", name=f"wg{ft}")
                (eng or nc.gpsimd).dma_start(wgt, wg[ft])
                wut = wpool.tile([P, c.KC, P], BF16, tag="w", name=f"wu{ft}")
                (eng or nc.gpsimd).dma_start(wut, wu[ft])
                return wgt, wut

            xT = xTp.tile([P, c.NMH, c.KC, c.MH], BF16, name="xT")

            def xld(eng, h, k0, k1):
                eng.dma_start(xT[:, h, k0:k1, :], xt[:, h, k0:k1, :])

            # PE warm-up: the HAM clock gate holds the array at 1.2 GHz
            # until it sees ~3.4us of sustained matmul activity, and the
            # first real operands only land ~8.5us in (DMA queue bootstrap).
            # A burst of dependency-free matmuls on a memset tile flips the
            # gate to 2.4 GHz during that dead window so every real matmul
            # issues warm.
            with tc.high_priority():
                wmt = consts.tile([P, 512], BF16, name="wmt")
                nc.vector.memset(wmt, 0.5)
                # borrow a buffer from the p1 rotation (it is idle until
                # the first real psum group) so no extra PSUM bank is used
                wps = pall.tile([P, 512], F32, tag="p1", bufs=2, name="wps")
                # engine preamble holds the PE until ~5.7us and the first
                # operands land ~8.8us: ~10 cold matmuls (427ns each) keep
                # the PE busy through that window without queueing ahead of
                # real work (PE queues are FIFO)
                NWARM = 10
                for i in range(NWARM):
                    nc.tensor.matmul(wps, wmt[:, :P], wmt,
                                     start=(i == 0), stop=(i == NWARM - 1))

            # Startup DMA schedule: the first gate/up f-tile's weights are
            # k-chunked and interleaved with a fine xT[h0] ladder across
            # BOTH HWDGE rings, ordered by first-use time, so the warm PE
            # can chew k-chunks as they land (~480 GB/s aggregate supply vs
            # 444 GB/s consumption). xT[h1] + ft1 weights follow the same
            # pattern; from ft2 weights ride SWDGE.
            with tc.high_priority():
                wg0 = wpool.tile([P, c.KC, P], BF16, tag="w", name="wg0")
                wu0 = wpool.tile([P, c.KC, P], BF16, tag="w", name="wu0")
                nc.scalar.dma_start(wg0[:, 0:2], wg[0][:, 0:2])
                nc.scalar.dma_start(wu0[:, 0:2], wu[0][:, 0:2])
                for k in (0, 1, 2):
                    xld(nc.sync, 0, k, k + 1)
                xld(nc.scalar, 0, 3, 4)
                nc.scalar.dma_start(wg0[:, 2:9], wg[0][:, 2:9])
                for k in (4, 5, 6):
                    xld(nc.sync, 0, k, k + 1)
                xld(nc.scalar, 0, 7, 8)
                nc.scalar.dma_start(wu0[:, 2:9], wu[0][:, 2:9])
                for k in (8, 9, 10):
                    xld(nc.sync, 0, k, k + 1)
                xld(nc.scalar, 0, 11, 12)
                nc.scalar.dma_start(wg0[:, 9:16], wg[0][:, 9:16])
                for k in (12, 13, 14):
                    xld(nc.sync, 0, k, k + 1)
                xld(nc.scalar, 0, 15, 16)
                nc.scalar.dma_start(wu0[:, 9:16], wu[0][:, 9:16])
                # h1 ladder split across the rings, ft1 weights woven in on
                # ACT so everything lands just ahead of first use
                wg1 = wpool.tile([P, c.KC, P], BF16, tag="w", name="wg1")
                wu1 = wpool.tile([P, c.KC, P], BF16, tag="w", name="wu1")
                xld(nc.sync, 1, 0, 2)
                xld(nc.scalar, 1, 2, 4)
                xld(nc.sync, 1, 4, 6)
                nc.scalar.dma_start(wg1[:, 0:8], wg[1][:, 0:8])
                xld(nc.scalar, 1, 6, 8)
                xld(nc.sync, 1, 8, 10)
                nc.scalar.dma_start(wg1[:, 8:16], wg[1][:, 8:16])
                xld(nc.scalar, 1, 10, 12)
                xld(nc.sync, 1, 12, 14)
                nc.scalar.dma_start(wu1[:, 0:8], wu[1][:, 0:8])
                xld(nc.scalar, 1, 14, 16)
                nc.scalar.dma_start(wu1[:, 8:16], wu[1][:, 8:16])
                pend = {0: (wg0, wu0), 1: (wg1, wu1)}
                bgt = consts.tile([P, c.NF], F32, name="bgt")
                nc.gpsimd.dma_start(bgt, bg[:, :])
                but = consts.tile([P, c.NF], F32, name="but")
                nc.gpsimd.dma_start(but, bu[:, :])
            bdf = consts.tile([P, c.D], F32, name="bdf")
            nc.gpsimd.dma_start(bdf, bd[:, :])

            # y accumulator, SBUF-resident across the 4 f-quarters
            yacc = yp.tile([P, c.NM, c.D], F32, name="yacc")

            DTAGS = ["p1", "p2", "pd0", "pd1"]
            for q in range(c.FQ):
                # ---- gate/up projections for this f-quarter ----
                x3 = [
                    x3p.tile([P, c.NT], BF16, tag=f"s{s}", name=f"x3_{q}_{s}")
                    for s in range(c.SQ)
                ]
                for s in range(c.SQ):
                    ft = q * c.SQ + s
                    wgt, wut = pend.pop(ft) if ft in pend else load_w(ft)
                    if ft + 2 < c.NF and ft + 2 not in pend:
                        pend[ft + 2] = load_w(ft + 2)
                    for h in range(c.NMH):
                        msl = slice(h * c.MH, (h + 1) * c.MH)
                        p1 = pall.tile([P, c.MH], F32, tag="p1", bufs=2,
                                       name=f"p1_{ft}_{h}")
                        p2 = pall.tile([P, c.MH], F32, tag="p2", bufs=2,
                                       name=f"p2_{ft}_{h}")
                        # gate/up matmuls interleaved per k-chunk: each xT
                        # chunk feeds two matmuls back-to-back, halving the
                        # supply rate the startup ladder has to sustain
                        for k in range(c.KC):
                            nc.tensor.matmul(
                                p1, wgt[:, k, :],
                                xT[:, h, k, :],
                                start=(k == 0), stop=(k == c.KC - 1))
                            nc.tensor.matmul(
                                p2, wut[:, k, :],
                                xT[:, h, k, :],
                                start=(k == 0), stop=(k == c.KC - 1))
                        t1 = actp.tile([P, c.MH], F32, tag="t1", name=f"t1_{ft}_{h}")
                        nc.scalar.activation(t1, p1, AF.Relu, bias=bgt[:, ft:ft + 1])
                        # x3 = (p2 + b_up) * relu(p1 + b_gate)
                        nc.vector.scalar_tensor_tensor(
                            x3[s][:, msl], p2, but[:, ft:ft + 1], t1,
                            op0=ALU.add, op1=ALU.mult)
                # ---- down projection partials for this f-quarter ----
                SB = 8  # wd stripes per DMA batch

                def evict(j, d, g, pd):
                    m = g * c.MG + j
                    dsl = slice(d * c.DC, (d + 1) * c.DC)
                    if q == 0:
                        # seed with b_down on the first partial
                        nc.vector.tensor_add(yacc[:, m, dsl], pd, bdf[:, dsl])
                    elif q < c.FQ - 1:
                        nc.vector.tensor_add(yacc[:, m, dsl], pd,
                                             yacc[:, m, dsl])
                    else:
                        ot = outp.tile([P, c.DC], F32, tag="ot",
                                       name=f"ot_{d}_{g}_{j}")
                        nc.vector.tensor_add(ot, pd, yacc[:, m, dsl])
                        # ACT's HWDGE ring is idle during the down phase —
                        # keeps writeback off the wd/weight rings. The very
                        # last chunks split across two rings so their HBM
                        # write-acks overlap.
                        eng = nc.sync if (q == c.FQ - 1 and d == c.ND - 1
                                          and g == c.NMG - 1
                                          and j % 2) else nc.scalar
                        eng.dma_start(y[m * P:(m + 1) * P, dsl], ot)

                for d in range(c.ND):
                    for g in range(c.NMG):
                        last = (q == c.FQ - 1 and d == c.ND - 1
                                and g == c.NMG - 1)
                        wdts = []
                        for s0 in range(0, c.SQ, SB):
                            wdt = wdpool.tile([P, SB, c.DC], BF16, tag="wd",
                                              name=f"wd_{q}_{d}_{g}_{s0}")
                            f0 = q * c.SQ + s0
                            nc.sync.dma_start(wdt, wd[d][:, f0:f0 + SB, :])
                            wdts.append(wdt)
                        # the final group runs as four single-psum sweeps so
                        # all but one eviction + writeback overlap later
                        # sweeps instead of trailing the kernel
                        jgroups = ([(0,), (1,), (2,)] if last
                                   else [tuple(range(c.MG))])
                        for jg in jgroups:
                            pds = {
                                j: pall.tile([P, c.DC], F32, tag=DTAGS[j],
                                             bufs=2, name=f"pd_{q}_{d}_{g}_{j}")
                                for j in jg
                            }
                            for s in range(c.SQ):
                                for j in jg:
                                    m = g * c.MG + j
                                    nc.tensor.matmul(
                                        pds[j],
                                        x3[s][:, m * P:(m + 1) * P],
                                        wdts[s // SB][:, s % SB, :],
                                        start=(s == 0),
                                        stop=(s == c.SQ - 1))
                            for j in jg:
                                evict(j, d, g, pds[j])
                        if last:
                            # very last m-chunk runs as two half-width psum
                            # sweeps: the first half's eviction + writeback
                            # overlaps the second half's matmuls, shortening
                            # the post-compute tail
                            j = c.MG - 1
                            m = g * c.MG + j
                            HC = c.DC // 2
                            for half in range(2):
                                pdt = pall.tile([P, c.DC], F32,
                                                tag=DTAGS[c.MG - 1], bufs=2,
                                                name=f"pdh_{half}")
                                pdh = pdt[:, :HC]
                                csl = slice(half * HC, (half + 1) * HC)
                                for s in range(c.SQ):
                                    nc.tensor.matmul(
                                        pdh,
                                        x3[s][:, m * P:(m + 1) * P],
                                        wdts[s // SB][:, s % SB, csl],
                                        start=(s == 0),
                                        stop=(s == c.SQ - 1))
                                dsl = slice(d * c.DC + half * HC,
                                            d * c.DC + (half + 1) * HC)
                                ot = outp.tile([P, HC], F32, tag="oth",
                                               name=f"oth_{half}")
                                nc.vector.tensor_add(ot, pdh, yacc[:, m, dsl])
                                eng = nc.sync if half else nc.scalar
                                eng.dma_start(y[m * P:(m + 1) * P, dsl], ot)

    nc.compile()
    return nc


def _prep_weights(w, a, b):
    """Fold LoRA into base weight (float64 accumulate, f32 round; the
    bf16 cast happens after the layout transpose in prep_inputs)."""
    weff = (w.astype(np.float64) + a.astype(np.float64) @ b.astype(np.float64))
    return weff.astype(np.float32)


def prep_inputs(inputs, cfg: Cfg):
    c = cfg
    x = np.asarray(inputs["x1"], np.float32).reshape(-1, c.D)
    n_tok = x.shape[0]
    assert n_tok == c.NT * c.N_CORES
    wg_e = _prep_weights(np.asarray(inputs["w_gate"], np.float32),
                         np.asarray(inputs["w_gate_lora_a"], np.float32),
                         np.asarray(inputs["w_gate_lora_b"], np.float32))
    wu_e = _prep_weights(np.asarray(inputs["w_up"], np.float32),
                         np.asarray(inputs["w_up_lora_a"], np.float32),
                         np.asarray(inputs["w_up_lora_b"], np.float32))
    wd_e = _prep_weights(np.asarray(inputs["w_down"], np.float32),
                         np.asarray(inputs["w_down_lora_a"], np.float32),
                         np.asarray(inputs["w_down_lora_b"], np.float32))
    # W[k_idx*P+kk, ft*P+ff] -> [ft, kk, k_idx, ff]
    wg_t = np.ascontiguousarray(
        wg_e.reshape(c.KC, P, c.NF, P).transpose(2, 1, 0, 3)).astype(NP_BF16)
    wu_t = np.ascontiguousarray(
        wu_e.reshape(c.KC, P, c.NF, P).transpose(2, 1, 0, 3)).astype(NP_BF16)
    # Wd[ft*P+ff, d*DC+dd] -> [d, ff, ft, dd]
    wd_t = np.ascontiguousarray(
        wd_e.reshape(c.NF, P, c.ND, c.DC).transpose(2, 1, 0, 3)).astype(NP_BF16)
    bg2 = np.ascontiguousarray(
        np.asarray(inputs["b_gate"], np.float32).reshape(c.NF, P).T)
    bu2 = np.ascontiguousarray(
        np.asarray(inputs["b_up"], np.float32).reshape(c.NF, P).T)
    bdf = np.ascontiguousarray(np.broadcast_to(
        np.asarray(inputs["b_down"], np.float32), (P, c.D)))
    in_maps = []
    for i in range(c.N_CORES):
        xs = x[i * c.NT:(i + 1) * c.NT]
        # [NT, D] -> [kk, h, k_idx, m']
        xt = np.ascontiguousarray(
            xs.T.reshape(c.KC, P, c.NMH, c.MH).transpose(1, 2, 0, 3)
        ).astype(NP_BF16)
        in_maps.append({
            "xt": xt,
            "wg": wg_t, "wu": wu_t, "wd": wd_t,
            "bg": bg2, "bu": bu2, "bd": bdf,
        })
    return in_maps


_CACHE = {}


def run(inputs, trace=False, trace_kwargs=None):
    cfg = Cfg()
    b, s, d = np.asarray(inputs["x1"]).shape
    in_maps = prep_inputs(inputs, cfg)
    key = "full"
    if key not in _CACHE:
        _CACHE[key] = build_bass(cfg)
    nc = _CACHE[key]
    res = run_bass_kernel_spmd(
        nc, in_maps, list(range(cfg.N_CORES)),
        trace=trace, **(trace_kwargs or {}))
    y = np.concatenate([res.results[i]["y"] for i in range(cfg.N_CORES)], axis=0)
    return y.reshape(b, s, d).astype(np.float32), res


def kernel(**inputs) -> np.ndarray:
    out, _ = run(inputs, trace=False)
    return out



# revision 10
# speedup vs baseline: 1.0033x; 1.0033x over previous
"""Trainium2 Bass kernel for a LoRA-augmented relu-gated MLP.

Math (per reference):
    y1 = x @ w_gate + b_gate + (x @ Ag) @ Bg
    y2 = x @ w_up   + b_up   + (x @ Au) @ Bu
    x3 = relu(y1) * y2
    y3 = x3 @ w_down + b_down + (x3 @ Ad) @ Bd

Strategy:
  * Host folds every LoRA pair into its base matrix (W_eff = W + A@B in
    float64) and rounds weights + activations to bf16 so the device kernel
    is a plain gated MLP running bf16 matmuls with f32 PSUM accumulation.
    bf16 stationary operands enable the PE's Fast Weight Load path, which
    hides the per-matmul LDWEIGHTS under the 512-column stream (fp32
    weights load in two passes and serialize ~60ns on every matmul).
  * Data parallel over the 8 NeuronCores: 8192 tokens -> 1024 per core,
    every core holds the full (folded) weights. Measured ~94% MFU; the
    matmul stream sits at the 216ns/512-column PE floor.
  * Per core the MLP is computed in f-quarters: gate/up produce x3T
    stripes [128f, NT] (bf16) in SBUF; the down projection consumes them
    as stationary operands and accumulates partial y3 into an SBUF-resident
    f32 accumulator; b_down is added on the first quarter's eviction. The
    last quarter streams finished y chunks straight to DRAM.
  * DMA streams are segregated by ring so none head-of-line-blocks
    another: xT + down-proj weights on the SP HWDGE ring, gate/up weight
    stream + biases on SWDGE, y writeback on the ACT HWDGE ring (idle
    during down phases). Small transfers are batched to keep per-partition
    lines >= 2KB, and the first xT batches use a staircase so the PE
    starts early and the HAM clock-gate warms without interruption.
"""

import sys
import types

import numpy as np

# The trimmed container's `antenv` lacks `axon_hooks`; bass_utils imports it
# unconditionally when tracing is requested (e.g. BASS_TRACE=1). Provide the
# degraded no-hook module so tracing falls back gracefully instead of crashing.
try:
    import antenv.axon_hooks  # noqa: F401
except ImportError:
    _m = types.ModuleType("antenv.axon_hooks")
    _m._hook = None
    _m.set_axon_ntff_profile_hook = lambda h: setattr(_m, "_hook", h)
    _m.get_axon_ntff_profile_hook = lambda: _m._hook
    sys.modules["antenv.axon_hooks"] = _m

import concourse.bacc as bacc
import concourse.bass as bass
import concourse.mybir as mybir
import concourse.tile as tile
from concourse.bass_utils import run_bass_kernel_spmd

P = 128
F32 = mybir.dt.float32
BF16 = mybir.dt.bfloat16
NP_BF16 = mybir.dt.np(mybir.dt.bfloat16)
AF = mybir.ActivationFunctionType
ALU = mybir.AluOpType


class Cfg:
    def __init__(self, nt=1024, d=2048, f=8192, fq=4, n_cores=8):
        assert nt % P == 0 and d % P == 0 and f % P == 0
        self.NT = nt          # tokens per core
        self.D = d            # model dim
        self.F = f            # ffn dim
        self.KC = d // P      # contraction chunks for gate/up
        self.NF = f // P      # f-tiles
        self.FQ = fq          # f quarters (x3T resident per quarter)
        assert self.NF % fq == 0
        self.SQ = self.NF // fq
        self.MH = min(512, nt)          # moving-dim chunk for gate/up
        self.NMH = nt // self.MH
        self.DC = min(512, d)           # down-proj d chunk
        self.ND = d // self.DC
        self.NM = nt // P               # token chunks of 128
        self.MG = 4                     # psum group size for down-proj
        self.NMG = self.NM // self.MG
        self.N_CORES = n_cores


def build_bass(cfg: Cfg):
    """Builds the per-core Bass program (same program on all cores)."""
    c = cfg
    nc = bacc.Bacc("TRN2", target_bir_lowering=False, debug=False,
                   num_swdge_queues=4)

    xt = nc.dram_tensor("xt", [P, c.NMH, c.KC, c.MH], BF16, kind="ExternalInput")
    wg = nc.dram_tensor("wg", [c.NF, P, c.KC, P], BF16, kind="ExternalInput")
    wu = nc.dram_tensor("wu", [c.NF, P, c.KC, P], BF16, kind="ExternalInput")
    wd = nc.dram_tensor("wd", [c.ND, P, c.NF, c.DC], BF16, kind="ExternalInput")
    bg = nc.dram_tensor("bg", [P, c.NF], F32, kind="ExternalInput")
    bu = nc.dram_tensor("bu", [P, c.NF], F32, kind="ExternalInput")
    bd = nc.dram_tensor("bd", [P, c.D], F32, kind="ExternalInput")
    y = nc.dram_tensor("y", [c.NT, c.D], F32, kind="ExternalOutput")

    with tile.TileContext(nc) as tc:
        with (
            tc.tile_pool(name="consts", bufs=1) as consts,
            tc.tile_pool(name="wpool", bufs=4) as wpool,
            tc.tile_pool(name="wdpool", bufs=3) as wdpool,
            tc.tile_pool(name="xTp", bufs=1) as xTp,
            tc.tile_pool(name="x3p", bufs=1) as x3p,
            tc.tile_pool(name="yp", bufs=1) as yp,
            tc.tile_pool(name="actp", bufs=2) as actp,
            tc.tile_pool(name="outp", bufs=6) as outp,
            tc.tile_pool(name="pall", bufs=1, space="PSUM") as pall,
        ):
            # the gate/up weight stream rides SWDGE (gpsimd) queues; the
            # SP HWDGE ring carries xT and the down-proj wd batches so
            # neither stream head-of-line-blocks the other. The first two
            # f-tiles' weights go on HWDGE rings instead (SWDGE's Q7 takes
            # several us to emit its first descriptors).
            def load_w(ft, eng=None):
                wgt = wpool.tile([P, c.KC, P], BF16, tag="w", name=f"wg{ft}")
                (eng or nc.gpsimd).dma_start(wgt, wg[ft])
                wut = wpool.tile([P, c.KC, P], BF16, tag="w", name=f"wu{ft}")
                (eng or nc.gpsimd).dma_start(wut, wu[ft])
                return wgt, wut

            # xT lives in three tiles so no tile has writers on more than
            # one DMA ring (cross-ring writes to one tile serialize issue):
            # h0 entirely on SP, h1 split k0:8 on SP / k8:16 on ACT.
            assert c.NMH == 2 and c.KC == 16
            KH = c.KC // 2
            xTa = xTp.tile([P, c.KC, c.MH], BF16, name="xTa")
            xTb1 = xTp.tile([P, KH, c.MH], BF16, name="xTb1")
            xTb2 = xTp.tile([P, KH, c.MH], BF16, name="xTb2")

            def xch(h, k):
                if h == 0:
                    return xTa[:, k, :]
                return xTb1[:, k, :] if k < KH else xTb2[:, k - KH, :]

            # PE warm-up: the HAM clock gate holds the array at 1.2 GHz
            # until it sees ~3.4us of sustained matmul activity, and the
            # first real operands only land ~8.5us in (DMA queue bootstrap).
            # A burst of dependency-free matmuls on a memset tile flips the
            # gate to 2.4 GHz during that dead window so every real matmul
            # issues warm.
            with tc.high_priority():
                wmt = consts.tile([P, 512], BF16, name="wmt")
                nc.vector.memset(wmt, 0.5)
                # borrow a buffer from the p1 rotation (it is idle until
                # the first real psum group) so no extra PSUM bank is used
                wps = pall.tile([P, 512], F32, tag="p1", bufs=2, name="wps")
                # engine preamble holds the PE until ~5.7us and the first
                # operands land ~8.8us: ~10 cold matmuls (427ns each) keep
                # the PE busy through that window without queueing ahead of
                # real work (PE queues are FIFO)
                NWARM = 10
                for i in range(NWARM):
                    nc.tensor.matmul(wps, wmt[:, :P], wmt,
                                     start=(i == 0), stop=(i == NWARM - 1))

            # Startup DMA schedule, arrival-ordered per ring and sized to
            # the measured ring rates (SP ~240 GB/s, ACT ~114, SWDGE ~73
            # burst). SP: xT[h0] ladder then xT[h1] k0:8. ACT: ft0 weights
            # in k-paired thirds (gate/up interleave consumes wg0[k] and
            # wu0[k] together) then xT[h1] k8:16 in quarters. SWDGE:
            # biases + ft1 weights, then the ft2+ stream. Few DMAs per
            # ring: the runtime only keeps ~11 in flight before issue
            # serializes on completions.
            with tc.high_priority():
                for k0, k1 in ((0, 2), (2, 4), (4, 8), (8, 12), (12, 16)):
                    nc.sync.dma_start(xTa[:, k0:k1, :], xt[:, 0, k0:k1, :])
                nc.sync.dma_start(xTb1, xt[:, 1, 0:KH, :])
                wg0 = wpool.tile([P, c.KC, P], BF16, tag="w", name="wg0")
                wu0 = wpool.tile([P, c.KC, P], BF16, tag="w", name="wu0")
                for k0, k1 in ((0, 6), (6, 11), (11, 16)):
                    nc.scalar.dma_start(wg0[:, k0:k1], wg[0][:, k0:k1])
                    nc.scalar.dma_start(wu0[:, k0:k1], wu[0][:, k0:k1])
                for k0 in range(0, KH, 2):
                    nc.scalar.dma_start(xTb2[:, k0:k0 + 2, :],
                                        xt[:, 1, KH + k0:KH + k0 + 2, :])
                bgt = consts.tile([P, c.NF], F32, name="bgt")
                nc.gpsimd.dma_start(bgt, bg[:, :])
                but = consts.tile([P, c.NF], F32, name="but")
                nc.gpsimd.dma_start(but, bu[:, :])
                pend = {0: (wg0, wu0), 1: load_w(1)}
            bdf = consts.tile([P, c.D], F32, name="bdf")
            nc.gpsimd.dma_start(bdf, bd[:, :])

            # y accumulator, SBUF-resident across the 4 f-quarters
            yacc = yp.tile([P, c.NM, c.D], F32, name="yacc")

            DTAGS = ["p1", "p2", "pd0", "pd1"]
            for q in range(c.FQ):
                # ---- gate/up projections for this f-quarter ----
                x3 = [
                    x3p.tile([P, c.NT], BF16, tag=f"s{s}", name=f"x3_{q}_{s}")
                    for s in range(c.SQ)
                ]
                for s in range(c.SQ):
                    ft = q * c.SQ + s
                    wgt, wut = pend.pop(ft) if ft in pend else load_w(ft)
                    if ft + 2 < c.NF and ft + 2 not in pend:
                        pend[ft + 2] = load_w(ft + 2)
                    for h in range(c.NMH):
                        msl = slice(h * c.MH, (h + 1) * c.MH)
                        p1 = pall.tile([P, c.MH], F32, tag="p1", bufs=2,
                                       name=f"p1_{ft}_{h}")
                        p2 = pall.tile([P, c.MH], F32, tag="p2", bufs=2,
                                       name=f"p2_{ft}_{h}")
                        # gate/up matmuls interleaved per k-chunk: each xT
                        # chunk feeds two matmuls back-to-back, halving the
                        # supply rate the startup ladder has to sustain
                        for k in range(c.KC):
                            nc.tensor.matmul(
                                p1, wgt[:, k, :],
                                xch(h, k),
                                start=(k == 0), stop=(k == c.KC - 1))
                            nc.tensor.matmul(
                                p2, wut[:, k, :],
                                xch(h, k),
                                start=(k == 0), stop=(k == c.KC - 1))
                        t1 = actp.tile([P, c.MH], F32, tag="t1", name=f"t1_{ft}_{h}")
                        nc.scalar.activation(t1, p1, AF.Relu, bias=bgt[:, ft:ft + 1])
                        # x3 = (p2 + b_up) * relu(p1 + b_gate)
                        nc.vector.scalar_tensor_tensor(
                            x3[s][:, msl], p2, but[:, ft:ft + 1], t1,
                            op0=ALU.add, op1=ALU.mult)
                # ---- down projection partials for this f-quarter ----
                SB = 8  # wd stripes per DMA batch

                def evict(j, d, g, pd):
                    m = g * c.MG + j
                    dsl = slice(d * c.DC, (d + 1) * c.DC)
                    if q == 0:
                        # seed with b_down on the first partial
                        nc.vector.tensor_add(yacc[:, m, dsl], pd, bdf[:, dsl])
                    elif q < c.FQ - 1:
                        nc.vector.tensor_add(yacc[:, m, dsl], pd,
                                             yacc[:, m, dsl])
                    else:
                        ot = outp.tile([P, c.DC], F32, tag="ot",
                                       name=f"ot_{d}_{g}_{j}")
                        nc.vector.tensor_add(ot, pd, yacc[:, m, dsl])
                        # ACT's HWDGE ring is idle during the down phase —
                        # keeps writeback off the wd/weight rings. The very
                        # last chunks split across two rings so their HBM
                        # write-acks overlap.
                        eng = nc.sync if (q == c.FQ - 1 and d == c.ND - 1
                                          and g == c.NMG - 1
                                          and j % 2) else nc.scalar
                        eng.dma_start(y[m * P:(m + 1) * P, dsl], ot)

                for d in range(c.ND):
                    for g in range(c.NMG):
                        last = (q == c.FQ - 1 and d == c.ND - 1
                                and g == c.NMG - 1)
                        wdts = []
                        for s0 in range(0, c.SQ, SB):
                            wdt = wdpool.tile([P, SB, c.DC], BF16, tag="wd",
                                              name=f"wd_{q}_{d}_{g}_{s0}")
                            f0 = q * c.SQ + s0
                            nc.sync.dma_start(wdt, wd[d][:, f0:f0 + SB, :])
                            wdts.append(wdt)
                        # the final group runs as four single-psum sweeps so
                        # all but one eviction + writeback overlap later
                        # sweeps instead of trailing the kernel
                        jgroups = ([(0,), (1,), (2,)] if last
                                   else [tuple(range(c.MG))])
                        for jg in jgroups:
                            pds = {
                                j: pall.tile([P, c.DC], F32, tag=DTAGS[j],
                                             bufs=2, name=f"pd_{q}_{d}_{g}_{j}")
                                for j in jg
                            }
                            for s in range(c.SQ):
                                for j in jg:
                                    m = g * c.MG + j
                                    nc.tensor.matmul(
                                        pds[j],
                                        x3[s][:, m * P:(m + 1) * P],
                                        wdts[s // SB][:, s % SB, :],
                                        start=(s == 0),
                                        stop=(s == c.SQ - 1))
                            for j in jg:
                                evict(j, d, g, pds[j])
                        if last:
                            # very last m-chunk runs as two half-width psum
                            # sweeps: the first half's eviction + writeback
                            # overlaps the second half's matmuls, shortening
                            # the post-compute tail
                            j = c.MG - 1
                            m = g * c.MG + j
                            HC = c.DC // 2
                            for half in range(2):
                                pdt = pall.tile([P, c.DC], F32,
                                                tag=DTAGS[c.MG - 1], bufs=2,
                                                name=f"pdh_{half}")
                                pdh = pdt[:, :HC]
                                csl = slice(half * HC, (half + 1) * HC)
                                for s in range(c.SQ):
                                    nc.tensor.matmul(
                                        pdh,
                                        x3[s][:, m * P:(m + 1) * P],
                                        wdts[s // SB][:, s % SB, csl],
                                        start=(s == 0),
                                        stop=(s == c.SQ - 1))
                                dsl = slice(d * c.DC + half * HC,
                                            d * c.DC + (half + 1) * HC)
                                ot = outp.tile([P, HC], F32, tag="oth",
                                               name=f"oth_{half}")
                                nc.vector.tensor_add(ot, pdh, yacc[:, m, dsl])
                                eng = nc.sync if half else nc.scalar
                                eng.dma_start(y[m * P:(m + 1) * P, dsl], ot)

    nc.compile()
    return nc


def _prep_weights(w, a, b):
    """Fold LoRA into base weight (float64 accumulate, f32 round; the
    bf16 cast happens after the layout transpose in prep_inputs)."""
    weff = (w.astype(np.float64) + a.astype(np.float64) @ b.astype(np.float64))
    return weff.astype(np.float32)


def prep_inputs(inputs, cfg: Cfg):
    c = cfg
    x = np.asarray(inputs["x1"], np.float32).reshape(-1, c.D)
    n_tok = x.shape[0]
    assert n_tok == c.NT * c.N_CORES
    wg_e = _prep_weights(np.asarray(inputs["w_gate"], np.float32),
                         np.asarray(inputs["w_gate_lora_a"], np.float32),
                         np.asarray(inputs["w_gate_lora_b"], np.float32))
    wu_e = _prep_weights(np.asarray(inputs["w_up"], np.float32),
                         np.asarray(inputs["w_up_lora_a"], np.float32),
                         np.asarray(inputs["w_up_lora_b"], np.float32))
    wd_e = _prep_weights(np.asarray(inputs["w_down"], np.float32),
                         np.asarray(inputs["w_down_lora_a"], np.float32),
                         np.asarray(inputs["w_down_lora_b"], np.float32))
    # W[k_idx*P+kk, ft*P+ff] -> [ft, kk, k_idx, ff]
    wg_t = np.ascontiguousarray(
        wg_e.reshape(c.KC, P, c.NF, P).transpose(2, 1, 0, 3)).astype(NP_BF16)
    wu_t = np.ascontiguousarray(
        wu_e.reshape(c.KC, P, c.NF, P).transpose(2, 1, 0, 3)).astype(NP_BF16)
    # Wd[ft*P+ff, d*DC+dd] -> [d, ff, ft, dd]
    wd_t = np.ascontiguousarray(
        wd_e.reshape(c.NF, P, c.ND, c.DC).transpose(2, 1, 0, 3)).astype(NP_BF16)
    bg2 = np.ascontiguousarray(
        np.asarray(inputs["b_gate"], np.float32).reshape(c.NF, P).T)
    bu2 = np.ascontiguousarray(
        np.asarray(inputs["b_up"], np.float32).reshape(c.NF, P).T)
    bdf = np.ascontiguousarray(np.broadcast_to(
        np.asarray(inputs["b_down"], np.float32), (P, c.D)))
    in_maps = []
    for i in range(c.N_CORES):
        xs = x[i * c.NT:(i + 1) * c.NT]
        # [NT, D] -> [kk, h, k_idx, m']
        xt = np.ascontiguousarray(
            xs.T.reshape(c.KC, P, c.NMH, c.MH).transpose(1, 2, 0, 3)
        ).astype(NP_BF16)
        in_maps.append({
            "xt": xt,
            "wg": wg_t, "wu": wu_t, "wd": wd_t,
            "bg": bg2, "bu": bu2, "bd": bdf,
        })
    return in_maps


_CACHE = {}


def run(inputs, trace=False, trace_kwargs=None):
    cfg = Cfg()
    b, s, d = np.asarray(inputs["x1"]).shape
    in_maps = prep_inputs(inputs, cfg)
    key = "full"
    if key not in _CACHE:
        _CACHE[key] = build_bass(cfg)
    nc = _CACHE[key]
    res = run_bass_kernel_spmd(
        nc, in_maps, list(range(cfg.N_CORES)),
        trace=trace, **(trace_kwargs or {}))
    y = np.concatenate([res.results[i]["y"] for i in range(cfg.N_CORES)], axis=0)
    return y.reshape(b, s, d).astype(np.float32), res


def kernel(**inputs) -> np.ndarray:
    out, _ = run(inputs, trace=False)
    return out

